# revision 1
# baseline (speedup 1.0000x reference)
"""Bahdanau-attention kernel for Trainium2 (8 NeuronCores, data-parallel over batch).

Computes, for each batch b:
    q[b]    = v * (W_w @ prev[b] + W_b + U_b)            (host, tiny)
    U'      = v[:, None] * U_w                            (host, tiny)
    e[b,t]  = sum_h relu(q[b,h] + (U' @ enc[b,t])_h)      (device)
    alpha   = softmax(e[b, :])                            (device)
    out[b]  = sum_t alpha[t] * enc[b,t,:]                 (device)

The v>0 fold is exact: v_h * relu(x_h) == relu(v_h * x_h) for v_h >= 0.

Device strategy (per core: 4 batches, enc slice [4, 4096, 1024] fp32 = 64 MB
streamed from HBM exactly once, cast fp32->fp16 during the DMA — fp16's
10-bit mantissa matches the tf32-grade rounding f32r gives on HW, at half
the byte width, 1 cyc/row PE transposes, and FWL fast weight loads):
  - enc tiles [t=128, c=1024] fp16 stay SBUF-resident for the batch.
  - PE transposes each tile chunk-wise to [c, t] (fp16, PSUM), DVE copies
    the result to SBUF.
  - U-matmul in fp16 accumulates [t=128, h=256] in fp32 PSUM on top of a
    ones-row x q bias matmul.
  - ACT fused relu+row-reduce produces the energy column per tile.
  - Exact fp32 two-level softmax: per-partition max shift via the ACT bias,
    then a one-partition fixup; cross-partition gather/scatter rides PE
    transposes / a K=1 matmul.
  - Pass-2 weighted sum: alpha column as stationary, natural enc tile as
    moving operand, accumulated into PSUM [1, 1024].

Toolchain notes: the module is built as a Bacc (not raw Bass) so multi-wait
instructions get legalized into event semaphores and the walrus single-wait
LDWEIGHTS limit is respected. Matmul inputs must not mix 16/32-bit dtypes;
the softmax's per-partition max is rounded to fp16 FIRST and the rounded
value used in both exponents so z'*g composes exactly.
"""

import sys

import numpy as np

sys.path.insert(0, "/opt/trn_rl_repo")

import concourse.bacc as bacc
import concourse.mybir as mybir
import concourse.tile as tile
from concourse.bass import ts
from concourse.bass_utils import run_bass_kernel_spmd
from concourse.masks import make_identity

B, T, C, H, D = 32, 4096, 1024, 256, 512
NCORES = 8
BPC = B // NCORES  # batches per core

F32 = mybir.dt.float32
F32R = mybir.dt.float32r
F16 = mybir.dt.float16
BF16 = mybir.dt.bfloat16

P = 128            # partitions / t-tile size
CK = C // P        # 8 c-chunks per tile
NT = T // P        # 32 t-tiles per batch


def build_bass(bpc: int = BPC, n_tiles: int = NT, repeat: int = 1):
    nc = bacc.Bacc(target_bir_lowering=False, trn_type="TRN2")

    enc = nc.dram_tensor("enc", [bpc, n_tiles * P, C], F32, kind="ExternalInput")
    # q rows packed on one partition: [1, bpc*H]
    qrow = nc.dram_tensor("qrow", [1, bpc * H], F32, kind="ExternalInput")
    # U' transposed, pre-arranged host-side as [p, chunk, h] with c = chunk*128 + p
    ut = nc.dram_tensor("ut", [P, CK, H], F32, kind="ExternalInput")
    out = nc.dram_tensor("out", [bpc, C], F32, kind="ExternalOutput")

    enc_ap = enc.ap()
    out_ap = out.ap()

    with tile.TileContext(nc) as tc:
        with (
            tc.tile_pool(name="singles", bufs=1) as singles,
            tc.tile_pool(name="enc_pool", bufs=n_tiles + 2) as enc_pool,
            tc.tile_pool(name="encT_pool", bufs=3) as encT_pool,
            tc.tile_pool(name="relu_pool", bufs=3) as relu_pool,
            tc.tile_pool(name="batch_pool", bufs=2) as batch_pool,
            tc.tile_pool(name="small_pool", bufs=2) as small_pool,
            tc.tile_pool(name="outst_pool", bufs=2) as outst_pool,
            tc.tile_pool(name="ps_tp", bufs=3, space="PSUM") as ps_tp,
            tc.tile_pool(name="ps_um", bufs=3, space="PSUM") as ps_um,
            tc.tile_pool(name="ps_c", bufs=1, space="PSUM") as ps_c,
        ):
            # --- constants, all funneled through DVE so PE sees one clock ---
            ident_stage = singles.tile([P, P], F32)
            make_identity(nc, ident_stage)
            ut_stage = singles.tile([P, CK, H], F32)
            nc.gpsimd.dma_start(out=ut_stage, in_=ut.ap())
            q_stage = singles.tile([1, bpc * H], F32)
            nc.gpsimd.dma_start(out=q_stage, in_=qrow.ap())

            ones_row_f = singles.tile([1, P], F32)
            nc.vector.memset(ones_row_f, 1.0)
            ones_row = singles.tile([1, P], F16)
            nc.vector.tensor_copy(ones_row, ones_row_f)
            q_s = singles.tile([1, bpc * H], F16)
            nc.vector.tensor_copy(q_s, q_stage)
            ut_s = singles.tile([P, CK, H], F16)
            nc.vector.tensor_copy(ut_s, ut_stage)
            ident_h = singles.tile([P, P], F16)
            nc.vector.tensor_copy(ident_h, ident_stage)

            def batches():
              for b in range(bpc):
                # ---------------- pass 1: energies ----------------
                enc_tiles = []
                e_buf = batch_pool.tile([P, n_tiles], F32, tag="ebuf")
                for j in range(n_tiles):
                    enc_t = enc_pool.tile([P, C], F16, tag="enc")
                    nc.gpsimd.dma_start(out=enc_t, in_=enc_ap[b, ts(j, P), :])
                    enc_tiles.append(enc_t)

                    # transpose per half: 4 chunks [t,c]->[c,t] into one
                    # PSUM bank, then one DVE copy [128, 512] to SBUF
                    encT = encT_pool.tile([P, C], F16, tag="encT")
                    tp = ps_tp.tile([P, C], F16, tag="tp")
                    for k in range(CK):
                        nc.tensor.transpose(
                            tp[:, ts(k, P)], enc_t[:, ts(k, P)], ident_h
                        )
                    nc.vector.tensor_copy(encT, tp)

                    # U-matmul: psum[t, h] = q[h] + sum_c encT[c,t]^T ut[c,h]
                    um = ps_um.tile([P, H], F32, tag="um")
                    nc.tensor.matmul(
                        um,
                        ones_row,
                        q_s[:, b * H : (b + 1) * H],
                        start=True,
                        stop=False,
                    )
                    for k in range(CK):
                        nc.tensor.matmul(
                            um,
                            encT[:, ts(k, P)],
                            ut_s[:, k, :],
                            start=False,
                            stop=(k == CK - 1),
                        )

                    # e[t] = sum_h relu(um[t, h])  (ACT, fused reduce)
                    relu_sc = relu_pool.tile([P, H], BF16, tag="relu")
                    nc.scalar.activation(
                        out=relu_sc,
                        in_=um,
                        func=mybir.ActivationFunctionType.Relu,
                        accum_out=e_buf[:, j : j + 1],
                    )

                # ------- softmax (exact fp32, two-level, PE transposes) -------
                # z'[p,j] = exp(e[p,j] - mp[p]) with the per-partition max mp
                # (ACT bias is per-partition, so no broadcast needed), then a
                # one-partition fixup computes g[p] = exp(mp[p]-M)/S and
                # alpha = z' * g  ==  exp(e-M)/S exactly. Cross-partition
                # gather/scatter rides the PE transpose (sub-us) instead of
                # SBUF->SBUF DMA (~1.5us fixed each).
                ms = small_pool.tile([P, 2], F32, tag="ms")
                nc.vector.tensor_reduce(
                    ms[:, 0:1], e_buf, axis=mybir.AxisListType.X,
                    op=mybir.AluOpType.max,
                )
                ms_r = small_pool.tile([P, 2], F16, tag="ms_r")
                nc.vector.tensor_copy(ms_r[:, 0:1], ms[:, 0:1])
                mpneg = small_pool.tile([P, 1], F32, tag="mpneg")
                nc.vector.tensor_scalar_mul(mpneg, ms_r[:, 0:1], -1.0)
                z = batch_pool.tile([P, n_tiles], F32, tag="z")
                nc.scalar.activation(
                    out=z,
                    in_=e_buf,
                    func=mybir.ActivationFunctionType.Exp,
                    bias=mpneg,
                    accum_out=ms[:, 1:2],
                )
                # gather each column onto partition 0 via PE transposes
                # (f32r rounding copies keep the BIR verifier happy)
                nc.vector.tensor_copy(ms_r[:, 1:2], ms[:, 1:2])
                mrow_ps = ps_tp.tile([1, P], F16, tag="tp")
                nc.tensor.transpose(mrow_ps, ms_r[:, 0:1], ident_h)
                srow_ps = ps_tp.tile([1, P], F16, tag="tp")
                nc.tensor.transpose(srow_ps, ms_r[:, 1:2], ident_h)
                mrow = small_pool.tile([1, P], F32, tag="mrow")
                nc.vector.tensor_copy(mrow, mrow_ps)
                srow = small_pool.tile([1, P], F32, tag="srow")
                nc.vector.tensor_copy(srow, srow_ps)
                mtot = small_pool.tile([1, 1], F32, tag="mtot")
                nc.vector.tensor_reduce(
                    mtot, mrow, axis=mybir.AxisListType.X, op=mybir.AluOpType.max
                )
                mtneg = small_pool.tile([1, 1], F32, tag="mtneg")
                nc.vector.tensor_scalar_mul(mtneg, mtot, -1.0)
                grow = small_pool.tile([1, P], F32, tag="grow")
                nc.scalar.activation(
                    out=grow,
                    in_=mrow,
                    func=mybir.ActivationFunctionType.Exp,
                    bias=mtneg,
                )
                wrow = small_pool.tile([1, P], F32, tag="wrow")
                nc.vector.tensor_mul(wrow, grow, srow)
                stot = small_pool.tile([1, 1], F32, tag="stot")
                nc.vector.tensor_reduce(
                    stot, wrow, axis=mybir.AxisListType.X, op=mybir.AluOpType.add
                )
                rec = small_pool.tile([1, 1], F32, tag="rec")
                nc.vector.reciprocal(rec, stot)
                gsrow = small_pool.tile([1, P], F32, tag="gsrow")
                nc.vector.tensor_scalar_mul(gsrow, grow, rec)
                gsrow_r = small_pool.tile([1, P], F16, tag="gsrow_r")
                nc.vector.tensor_copy(gsrow_r, gsrow)
                # scatter g[p]/S back to one element per partition via a
                # K=1 matmul: out[p, 0] = gsrow[p] * 1
                gscol_ps = ps_tp.tile([P, 32], F32, tag="tp")
                nc.tensor.matmul(
                    gscol_ps, gsrow_r, ones_row[:, 0:32], start=True, stop=True
                )
                gscol = small_pool.tile([P, 1], F32, tag="gscol")
                nc.vector.tensor_copy(gscol, gscol_ps[:, 0:1])
                alpha = batch_pool.tile([P, n_tiles], F16, tag="alpha")
                nc.vector.tensor_scalar_mul(alpha, z, gscol)

                # ---------------- pass 2: weighted sum ----------------
                cps = ps_c.tile([1, 2, D], F32, tag="cps")
                for j in range(n_tiles):
                    for h in range(2):
                        nc.tensor.matmul(
                            cps[:, h, :],
                            alpha[:, j : j + 1],
                            enc_tiles[j][:, ts(h, D)],
                            start=(j == 0),
                            stop=(j == n_tiles - 1),
                        )
                c_st = outst_pool.tile([1, C], F32, tag="cst")
                nc.vector.tensor_copy(c_st, cps.rearrange("p a b -> p (a b)"))
                nc.sync.dma_start(out=out_ap[b : b + 1, :], in_=c_st)

            if repeat == 1:
                batches()
            else:
                with tc.For_i(0, repeat, 1):
                    batches()

    return nc


_NC_CACHE: dict = {}


def _get_nc(bpc=BPC, n_tiles=NT):
    key = (bpc, n_tiles)
    if key not in _NC_CACHE:
        nc = build_bass(bpc, n_tiles)
        if not nc.is_finalized():
            nc.finalize()
        _NC_CACHE[key] = nc
    return _NC_CACHE[key]


def _host_prep(previous_decoder_hidden_state, W_w, W_b, U_w, U_b, v):
    prev = np.asarray(previous_decoder_hidden_state, dtype=np.float32)[:, 0, :]
    W_w = np.asarray(W_w, dtype=np.float32)
    U_w = np.asarray(U_w, dtype=np.float32)
    v = np.asarray(v, dtype=np.float32)
    bias = np.asarray(W_b, dtype=np.float32) + np.asarray(U_b, dtype=np.float32)
    q_all = (v[None, :] * (prev @ W_w.T + bias)).astype(np.float32)  # [B, H]
    up = (v[:, None] * U_w).astype(np.float32)  # [H, C]
    # ut_host[p, k, h] = up.T[k*128 + p, h]
    ut_host = np.ascontiguousarray(up.T.reshape(CK, P, H).transpose(1, 0, 2))
    return q_all, ut_host


def kernel(**inputs) -> np.ndarray:
    enc = np.ascontiguousarray(
        np.asarray(inputs["encoder_final_hidden_layers"], dtype=np.float32)
    )
    q_all, ut_host = _host_prep(
        inputs["previous_decoder_hidden_state"],
        inputs["W_w"],
        inputs["W_b"],
        inputs["U_w"],
        inputs["U_b"],
        inputs["v"],
    )

    nc = _get_nc()
    in_maps = []
    for i in range(NCORES):
        sl = slice(i * BPC, (i + 1) * BPC)
        in_maps.append(
            {
                "enc": enc[sl],
                "qrow": np.ascontiguousarray(q_all[sl].reshape(1, BPC * H)),
                "ut": ut_host,
            }
        )
    try:
        res = run_bass_kernel_spmd(nc, in_maps, core_ids=list(range(NCORES)))
    except Exception:
        # a previously crashed run can leave a core wedged
        # (NRT_EXEC_UNIT_UNRECOVERABLE); one retry recovers
        res = run_bass_kernel_spmd(nc, in_maps, core_ids=list(range(NCORES)))
    return np.concatenate([r["out"] for r in res.results], axis=0)


if __name__ == "__main__":
    nc = build_bass()
    print("built ok")



# revision 2
# speedup vs baseline: 1.0845x; 1.0845x over previous
"""Bahdanau-attention kernel for Trainium2 (8 NeuronCores, data-parallel over batch).

Single-pass ONLINE-softmax design. For each batch b (4 per core):
    q[b]    = v * (W_w @ prev[b] + W_b + U_b)           (host, tiny)
    U'      = v[:, None] * U_w                          (host, tiny)
    shift_b = sum_h relu(q[b,h])                        (host; ~ E[e])
    e[t]    = sum_h relu(q[b,h] + (enc[b,t] @ U')_h)    (device)
    z[t]    = exp(e[t] - shift_b)    (bf16; exact softmax - shift cancels)
    C[c]    = sum_t z[t] * enc[b,t,c];  S = sum_t z[t]  (device, streamed)
    out[b]  = C / S

Key points:
  - No global softmax pass: z uses a FIXED per-batch shift, so each tile's
    weighted contribution accumulates into PSUM immediately (PSUM group
    start/stop across the 32 tiles) and enc tiles are freed right away.
    z is bf16: the 8-bit exponent makes overflow impossible (|e - shift|
    would have to reach ~88; measured spread on the reference inputs is
    [-6.5, +47]).  Energies stay fp16-precise; softmax z/S is exact math.
  - enc streamed ONCE from HBM (64 MB f32 per core), f32->fp16 cast in the
    SWDGE DMA.  t-index permuted so each partition holds 4 CONSECUTIVE
    rows (t = grp*512 + 4p + i): per-subtile DMAs with contiguous 4 KB
    descriptors; 24-deep tile pool gives a ~20-tile prefetch window.
    The t-permutation is free: softmax is over all t, and z pairs with
    enc in the same layout.
  - q-bias applied by DVE (um += qb) instead of a PE bias matmul.
  - PE per tile: 8 fp16 transposes (enc has t on partitions, the U-matmul
    contracts c, so both layouts are needed), 8 U-matmuls (encT stationary,
    ut moving, fp32 PSUM), 2 weighted-sum matmuls + 1 S-matmul with the
    z column stationary (1-col LDWEIGHTS, ~free).
  - Pipelined issue order per slot s: TP(s) | U(s-1) | P2(s-1-LAG): the
    one-slot lag hides the DVE encT copy, LAG=2 hides the ACT relu/exp
    chain, so PE (the bottleneck engine at ~93% occupancy in CoreSim)
    never stalls on same-tile dependencies.  PE stalls are doubly bad on
    TRN2: the PE p-state drops to 1.2 GHz (from 2.4) after any idle and
    needs 3 us of continuous work to ramp back.

Measured (in one session, repeat-loop slope): baseline two-pass 365 us ->
this kernel 293 us; CoreSim cost model: 227 us (PE-busy 220 us of it);
HBM floor 64 MB @ 375 GB/s measured = 171 us.
"""

import sys

import numpy as np

sys.path.insert(0, "/opt/trn_rl_repo")

import concourse.bacc as bacc
import concourse.mybir as mybir
import concourse.tile as tile
from concourse.bass import ts
from concourse.bass_utils import run_bass_kernel_spmd
from concourse.masks import make_identity

B, T, C, H, D = 32, 4096, 1024, 256, 512
NCORES = 8
BPC = B // NCORES  # batches per core

F32 = mybir.dt.float32
F16 = mybir.dt.float16
BF16 = mybir.dt.bfloat16

P = 128            # partitions / t-tile size
CK = C // P        # 8 c-chunks per tile
NT = T // P        # 32 t-tiles per batch
GT = 4             # tiles per DMA group (2 MB reads)
LAG = 2            # extra tiles of slack before the pass-2 use of z


def build_bass(bpc: int = BPC, n_tiles: int = NT, repeat: int = 1, sweeps: int = 1):
    nc = bacc.Bacc(target_bir_lowering=False, trn_type="TRN2")
    ngrp = n_tiles // GT

    enc = nc.dram_tensor("enc", [bpc, ngrp, P, GT, C], F32, kind="ExternalInput")
    qb = nc.dram_tensor("qb", [P, bpc, H], F32, kind="ExternalInput")
    # U' transposed, host-side arranged as [p, chunk, h] with c = chunk*128 + p
    ut = nc.dram_tensor("ut", [P, CK, H], F32, kind="ExternalInput")
    # -shift_b replicated across partitions (ACT bias is per-partition)
    shifts = nc.dram_tensor("shifts", [P, bpc], F32, kind="ExternalInput")
    out = nc.dram_tensor("out", [bpc, C], F32, kind="ExternalOutput")

    enc_ap = enc.ap()
    out_ap = out.ap()

    with tile.TileContext(nc) as tc:
        with (
            tc.tile_pool(name="singles", bufs=1) as singles,
            tc.tile_pool(name="enc_pool", bufs=24) as enc_pool,
            tc.tile_pool(name="encT_pool", bufs=3) as encT_pool,
            tc.tile_pool(name="relu_pool", bufs=3) as relu_pool,
            tc.tile_pool(name="ecol_pool", bufs=6) as ecol_pool,
            tc.tile_pool(name="zcol_pool", bufs=6) as zcol_pool,
            tc.tile_pool(name="outst_pool", bufs=2) as outst_pool,
            tc.tile_pool(name="ps_tp", bufs=2, space="PSUM") as ps_tp,
            tc.tile_pool(name="ps_um", bufs=3, space="PSUM") as ps_um,
            tc.tile_pool(name="ps_c", bufs=1, space="PSUM") as ps_c,
            tc.tile_pool(name="ps_s", bufs=1, space="PSUM") as ps_s,
        ):
            # --- constants ---
            ident_stage = singles.tile([P, P], F32)
            make_identity(nc, ident_stage)
            ut_stage = singles.tile([P, CK, H], F32)
            nc.sync.dma_start(out=ut_stage, in_=ut.ap())
            qb_s = singles.tile([P, bpc, H], F32)
            nc.sync.dma_start(out=qb_s, in_=qb.ap())
            shifts_s = singles.tile([P, bpc], F32)
            nc.sync.dma_start(out=shifts_s, in_=shifts.ap())

            ones_col_f = singles.tile([P, 1], F32)
            nc.vector.memset(ones_col_f, 1.0)
            ones_col = singles.tile([P, 1], F16)
            nc.vector.tensor_copy(ones_col, ones_col_f)
            ut_s = singles.tile([P, CK, H], F16)
            nc.vector.tensor_copy(ut_s, ut_stage)
            ident_h = singles.tile([P, P], F16)
            nc.vector.tensor_copy(ident_h, ident_stage)

            def run():
                total = bpc * n_tiles
                enc_groups = {}  # grp index -> [P, GT, C] fp16 tile
                enc_views = {}   # g -> [P, C] natural view
                encTs = {}       # g -> [P, C] fp16 transposed tile
                ums = {}         # g -> [P, H] f32 psum
                zcols = {}       # g -> [P, 1] bf16
                cps_t = {}       # b -> [1, 2, D] f32 psum
                sps_t = {}       # b -> [1, 1] f32 psum

                def stage_tp(g):
                    b, j = divmod(g, n_tiles)
                    grp, gi = divmod(j, GT)
                    et = enc_pool.tile([P, C], F16, tag="enc", name="enc_t")
                    nc.gpsimd.dma_start(out=et, in_=enc_ap[b, grp, :, gi])
                    enc_views[g] = et
                    enc_t = et
                    tp = ps_tp.tile([P, C], F16, tag="tp")
                    for k in range(CK):
                        nc.tensor.transpose(
                            tp[:, ts(k, P)], enc_t[:, ts(k, P)], ident_h
                        )
                    encT = encT_pool.tile([P, C], F16, tag="encT")
                    nc.vector.tensor_copy(encT, tp)
                    encTs[g] = encT

                def stage_u(g):
                    b, j = divmod(g, n_tiles)
                    encT = encTs.pop(g)
                    um = ps_um.tile([P, H], F32, tag="um")
                    for k in range(CK):
                        nc.tensor.matmul(
                            um,
                            encT[:, ts(k, P)],
                            ut_s[:, k, :],
                            start=(k == 0),
                            stop=(k == CK - 1),
                        )
                    nc.vector.tensor_add(um, um, qb_s[:, b, :])
                    relu_sc = relu_pool.tile([P, H], BF16, tag="relu")
                    e_col = ecol_pool.tile([P, 1], F32, tag="ecol")
                    nc.scalar.activation(
                        out=relu_sc,
                        in_=um,
                        func=mybir.ActivationFunctionType.Relu,
                        accum_out=e_col,
                    )
                    z_col = zcol_pool.tile([P, 1], BF16, tag="zcol")
                    nc.scalar.activation(
                        out=z_col,
                        in_=e_col,
                        func=mybir.ActivationFunctionType.Exp,
                        bias=shifts_s[:, b : b + 1],
                    )
                    zcols[g] = z_col

                def stage_p2(g):
                    b, j = divmod(g, n_tiles)
                    z_col = zcols.pop(g)
                    if j == 0:
                        cps_t[b] = ps_c.tile([1, 2, D], F32, tag="cps", name="cps")
                        sps_t[b] = ps_s.tile([1, 1], F32, tag="sps", name="sps")
                    cps = cps_t[b]
                    sps = sps_t[b]
                    enc_t = enc_views.pop(g)
                    last = j == n_tiles - 1
                    for h in range(2):
                        nc.tensor.matmul(
                            cps[:, h, :],
                            z_col,
                            enc_t[:, ts(h, D)],
                            start=(j == 0),
                            stop=last,
                        )
                    nc.tensor.matmul(
                        sps, z_col, ones_col, start=(j == 0), stop=last
                    )
                    if last:
                        rec = outst_pool.tile([1, 1], F32, tag="rec")
                        nc.vector.reciprocal(rec, sps_t.pop(b))
                        c_st = outst_pool.tile([1, C], F32, tag="cst")
                        nc.vector.tensor_scalar_mul(
                            c_st, cps_t.pop(b).rearrange("p a b -> p (a b)"), rec
                        )
                        nc.sync.dma_start(out=out_ap[b : b + 1, :], in_=c_st)

                for s in range(total + 1 + LAG):
                    if s < total:
                        stage_tp(s)
                    if 0 <= s - 1 < total:
                        stage_u(s - 1)
                    if 0 <= s - 1 - LAG < total:
                        stage_p2(s - 1 - LAG)

            if repeat == 1:
                for _ in range(sweeps):
                    run()
            else:
                with tc.For_i(0, repeat, 1):
                    for _ in range(sweeps):
                        run()

    return nc


_NC_CACHE: dict = {}


def _get_nc(bpc=BPC, n_tiles=NT):
    key = (bpc, n_tiles)
    if key not in _NC_CACHE:
        nc = build_bass(bpc, n_tiles)
        if not nc.is_finalized():
            nc.finalize()
        _NC_CACHE[key] = nc
    return _NC_CACHE[key]


def _host_prep(previous_decoder_hidden_state, W_w, W_b, U_w, U_b, v):
    prev = np.asarray(previous_decoder_hidden_state, dtype=np.float32)[:, 0, :]
    W_w = np.asarray(W_w, dtype=np.float32)
    U_w = np.asarray(U_w, dtype=np.float32)
    v = np.asarray(v, dtype=np.float32)
    bias = np.asarray(W_b, dtype=np.float32) + np.asarray(U_b, dtype=np.float32)
    q_all = (v[None, :] * (prev @ W_w.T + bias)).astype(np.float32)  # [B, H]
    up = (v[:, None] * U_w).astype(np.float32)  # [H, C]
    # ut_host[p, k, h] = up.T[k*128 + p, h]
    ut_host = np.ascontiguousarray(up.T.reshape(CK, P, H).transpose(1, 0, 2))
    shift_all = np.clip(q_all, 0.0, None).sum(axis=1)  # [B]
    return q_all, ut_host, shift_all




def _in_maps(enc, q_all, ut_host, shift_all):
    in_maps = []
    for i in range(NCORES):
        sl = slice(i * BPC, (i + 1) * BPC)
        in_maps.append(
            {
                "enc": enc[sl].reshape(BPC, NT // GT, P, GT, C),
                "qb": np.ascontiguousarray(
                    np.broadcast_to(q_all[sl][None, :, :], (P, BPC, H))
                ).astype(np.float32),
                "ut": ut_host,
                "shifts": np.ascontiguousarray(
                    np.broadcast_to(-shift_all[sl][None, :], (P, BPC))
                ).astype(np.float32),
            }
        )
    return in_maps


def make_in_maps(inputs):
    enc = np.ascontiguousarray(
        np.asarray(inputs["encoder_final_hidden_layers"], dtype=np.float32)
    )
    q_all, ut_host, shift_all = _host_prep(
        inputs["previous_decoder_hidden_state"],
        inputs["W_w"], inputs["W_b"], inputs["U_w"], inputs["U_b"], inputs["v"],
    )
    return _in_maps(enc, q_all, ut_host, shift_all)

def kernel(**inputs) -> np.ndarray:
    enc = np.ascontiguousarray(
        np.asarray(inputs["encoder_final_hidden_layers"], dtype=np.float32)
    )
    q_all, ut_host, shift_all = _host_prep(
        inputs["previous_decoder_hidden_state"],
        inputs["W_w"],
        inputs["W_b"],
        inputs["U_w"],
        inputs["U_b"],
        inputs["v"],
    )

    nc = _get_nc()
    in_maps = _in_maps(enc, q_all, ut_host, shift_all)
    try:
        res = run_bass_kernel_spmd(nc, in_maps, core_ids=list(range(NCORES)))
    except Exception:
        res = run_bass_kernel_spmd(nc, in_maps, core_ids=list(range(NCORES)))
    return np.concatenate([r["out"] for r in res.results], axis=0)


if __name__ == "__main__":
    nc = build_bass()
    print("built ok")
